# revision 15
# baseline (speedup 1.0000x reference)
"""Trainium2 Bass kernel for nn_DirectionalConv (moe_routing).

Math: out = (1/8) * sum_k conv3x3(x * [octant(sobel(x)) == k], W[k]) + mean_k b[k]

Implementation notes:
- Data-parallel over batch B=8 across 8 NeuronCores (one image per core).
- Octant selection is rewritten in a +-1 "monomial" basis over the three sign
  bits (sign(gy), sign(gx), sign(|gy|-|gx|)):
      sum_k conv(x*mask_k, W[k]) = sum_{S in 2^3} conv(x*chi_S, W'_S)
  where chi_S = product of the selected signs (computed with pure bitwise
  XOR of sign bits - exact) and W'_S = (1/64) sum_k chi_S(k) W[k] is
  precomputed on the host.  This gives 8 dense 3x3 convs, evaluated as
  9 shifted matmuls each, accumulating in PSUM.
- Per-core image (64,256,256) is split into top/bottom halves across the
  SBUF partition dimension: partition p = (half<<6)|channel.  Conv matmuls
  are K=64 and use 4-way PE tile packing (2 row-groups x 2 col-groups) to
  fill the 128x128 array.
- PSUM is DMAed straight to HBM (bias is added on the host), so neither ACT
  nor DVE ever waits on PE progress; out-DMAs are emitted one chunk late so
  no DMA descriptor ever parks in a queue blocking the x prefetch.
- Emission is software-pipelined: at/ut/mono0 of chunk i+1 are emitted in
  body i so the DVE FIFO has fill work while ACT produces the sign casts.
- Gradient tensors' border columns are never zeroed: only their sign bit is
  consumed, and it only flips the sign of an (exactly zero) padding value.
"""

import numpy as np

import concourse.bacc as bacc
import concourse.bass as bass
import concourse.mybir as mybir
from concourse import bass_utils
from concourse.tile import TileContext

F32 = mybir.dt.float32
F8 = mybir.dt.float8e4
F16 = mybir.dt.float16
U32 = mybir.dt.uint32
ALU = mybir.AluOpType
ACTF = mybir.ActivationFunctionType

B, C, H, W_, K, O = 8, 64, 256, 256, 8, 64
HH = H // 2          # rows per half
R = 8                # output rows per half per chunk
NCHUNK = HH // R     # 16
RG = R + 2           # gradient rows per chunk
WP = W_ + 2          # padded width 258
SIGN16 = 0x80008000  # sign bits of two packed fp16 lanes

MORDER = (0, 2, 4, 6, 1, 3, 5, 7)  # monomial readiness order


def _build_nc():
    nc = bacc.Bacc("TRN2", target_bir_lowering=False, debug=False)

    x_d = nc.dram_tensor("x", [C, H, W_], F32, kind="ExternalInput")
    wt_d = nc.dram_tensor("wt", [128, 8, 9, O], F16, kind="ExternalInput")
    out_d = nc.dram_tensor("out", [O, H, W_], F16, kind="ExternalOutput")

    with TileContext(nc) as tc:
        with (
            tc.tile_pool(name="sb", bufs=1) as sb,
            tc.tile_pool(name="pp", bufs=1, space="PSUM") as pp,
        ):
            # ---- static tiles
            wt = sb.tile([128, 8, 9, O], F16, tag="wt")
            wt8 = None
            maskT = sb.tile([128, 1], U32, tag="maskT")
            nc.gpsimd.memset(maskT[:], SIGN16)

            # ---- rotating buffers (explicit, so border memsets hoist)
            xts = [sb.tile([128, R + 4, WP], F32, tag=f"xt{i}", name=f"xt{i}")
                   for i in range(3)]
            for xt in xts:
                nc.gpsimd.memset(xt[:, :, 0:1], 0.0)
                nc.gpsimd.memset(xt[:, :, WP - 1:WP], 0.0)
            monos = [sb.tile([128, 8, RG, WP], F16, tag=f"mono{i}",
                             name=f"mono{i}") for i in range(2)]
            at_t = sb.tile([128, RG, WP], F32, tag="at")
            tt_t = sb.tile([128, RG, WP], F32, tag="tt")
            ut_t = sb.tile([128, RG, WP], F32, tag="ut")
            gx_t = sb.tile([128, RG, WP], F32, tag="gx")
            gy_t = sb.tile([128, RG, WP], F32, tag="gy")
            gxh = sb.tile([128, RG, WP], F16, tag="gxh")
            gyh = sb.tile([128, RG, WP], F16, tag="gyh")
            eh = sb.tile([128, RG, WP], F16, tag="eh")
            stgs = [sb.tile([128, 1024], F16, tag=f"stg{i}", name=f"stg{i}")
                    for i in range(2)]

            pss = [pp.tile([128, 1024], F32, tag=f"ps{i}", name=f"ps{i}")
                   for i in range(4)]

            def dma_x(ci):
                xt = xts[ci % 3]
                r0 = ci * R
                tlo, thi = r0 - 2, r0 + R + 2
                if tlo < 0:
                    nc.gpsimd.memset(xt[0:64, 0:-tlo, 1:WP - 1], 0.0)
                    nc.sync.dma_start(xt[0:64, -tlo:R + 4, 1:WP - 1],
                                      x_d[:, 0:thi, :])
                else:
                    nc.sync.dma_start(xt[0:64, :, 1:WP - 1], x_d[:, tlo:thi, :])
                blo, bhi = HH + r0 - 2, HH + r0 + R + 2
                if bhi > H:
                    nval = H - blo
                    nc.gpsimd.memset(xt[64:128, nval:R + 4, 1:WP - 1], 0.0)
                    nc.sync.dma_start(xt[64:128, 0:nval, 1:WP - 1],
                                      x_d[:, blo:H, :])
                else:
                    nc.sync.dma_start(xt[64:128, :, 1:WP - 1], x_d[:, blo:bhi, :])

            def lead_ops(ci):
                """chunk ci's chain head: emitted one body early (sw pipeline).
                All xt readers live here so the next x prefetch DMA's WAR
                releases a full chunk early.  For ci>0, gradient/monomial rows
                0..1 equal the previous chunk's rows 8..9 (same image rows):
                they are carried over by one ACT copy and only rows 2.. are
                computed."""
                xt = xts[ci % 3]
                g0 = 0 if ci == 0 else 2
                nc.vector.tensor_add(at_t[:, g0:RG], xt[:, g0:RG, :],
                                     xt[:, g0 + 2:RG + 2, :])
                nc.vector.tensor_sub(ut_t[:, g0:RG], xt[:, g0:RG, :],
                                     xt[:, g0 + 2:RG + 2, :])
                nc.vector.scalar_tensor_tensor(tt_t[:, g0:RG],
                                               xt[:, g0 + 1:RG + 1, :], 2.0,
                                               at_t[:, g0:RG], ALU.mult, ALU.add)
                nc.scalar.activation(monos[ci % 2][:, 0, g0:RG],
                                     xt[:, g0 + 1:RG + 1, :], ACTF.Copy)

            def evac(ps, y0):
                # ACT-issued DMAs (the second hardware DGE queue): the x
                # prefetch queue (SP) must never park behind PE-gated
                # descriptors, or the whole pipeline latency-locks.
                stg = stgs[(y0 // 4) % 2]
                nc.scalar.activation(stg[:], ps[:], ACTF.Copy)
                yb = HH + y0
                nc.gpsimd.dma_start(out_d[:, y0:y0 + 2, :], stg[0:64, 0:512])
                nc.gpsimd.dma_start(out_d[:, y0 + 2:y0 + 4, :], stg[64:128, 0:512])
                nc.gpsimd.dma_start(out_d[:, yb:yb + 2, :], stg[0:64, 512:1024])
                nc.gpsimd.dma_start(out_d[:, yb + 2:yb + 4, :], stg[64:128, 512:1024])

            dma_x(0)
            nc.sync.dma_start(wt[:], wt_d[:])
            dma_x(1)
            lead_ops(0)
            pending = []  # 2-chunk-delayed PSUM evacuations: (ps, y0)

            for ci in range(NCHUNK):
                if ci + 2 < NCHUNK:
                    dma_x(ci + 2)
                xt = xts[ci % 3]
                mono = monos[ci % 2]
                r0 = ci * R

                # ---- evacuate the 2-chunks-ago PSUM banks at the body top:
                # by the time ACT reaches these in FIFO order their PE deps
                # are long satisfied, and PE's bank reuse (WAR) is met a full
                # chunk before it matters.
                if ci >= 2:
                    for _ in range(2):
                        ps, y0 = pending.pop(0)
                        evac(ps, y0)

                g0 = 0 if ci == 0 else 2
                # ---- carry monomial rows 0..1 from the previous chunk's
                # buffer (same image rows as its rows 8..9): one ACT copy for
                # all 8 monomials.
                if ci > 0:
                    nc.scalar.activation(mono[:, :, 0:2, :],
                                         monos[(ci - 1) % 2][:, :, R:RG, :],
                                         ACTF.Copy)

                # ---- gx chain (DVE + ACT cast)
                nc.vector.tensor_tensor(gx_t[:, g0:RG, 1:WP - 1],
                                        tt_t[:, g0:RG, 0:WP - 2],
                                        tt_t[:, g0:RG, 2:WP], ALU.subtract)
                nc.scalar.activation(gxh[:, g0:RG], gx_t[:, g0:RG], ACTF.Copy)

                # ---- gy chain; b2 reuses at's memory (at consumed by tt)
                b2 = at_t[:, g0:RG, 0:W_]
                nc.vector.tensor_add(b2, ut_t[:, g0:RG, 0:WP - 2],
                                     ut_t[:, g0:RG, 2:WP])
                nc.vector.scalar_tensor_tensor(gy_t[:, g0:RG, 1:WP - 1],
                                               ut_t[:, g0:RG, 1:WP - 1], 2.0, b2,
                                               ALU.mult, ALU.add)
                nc.scalar.activation(gyh[:, g0:RG], gy_t[:, g0:RG], ACTF.Copy)

                # ---- e-sign via gy^2-gx^2 (same sign as |gy|-|gx|).
                # ay reuses tt's memory, ax reuses at's (b2 is dead after gy).
                ay = tt_t[:, g0:RG, 0:W_]
                nc.scalar.activation(ay, gy_t[:, g0:RG, 1:WP - 1], ACTF.Square)
                ax = at_t[:, g0:RG, 0:W_]
                nc.scalar.activation(ax, gx_t[:, g0:RG, 1:WP - 1], ACTF.Square)

                # ---- e into gx's memory (only sign bits survive)
                nc.vector.tensor_tensor(gx_t[:, g0:RG, 1:WP - 1], ay, ax,
                                        ALU.subtract)
                nc.scalar.activation(eh[:, g0:RG], gx_t[:, g0:RG], ACTF.Copy)

                # ---- next chunk's chain head: emitted after e32 so the WAR on
                # ay (living in tt's tile) is tracked; the engine wait-queue
                # lets these bypass a parked e32 and fill the DVE bubble.
                if ci + 1 < NCHUNK:
                    lead_ops(ci + 1)

                # ---- monomials y_S = x * chi_S, S = (sy<<2)|(sx<<1)|sd
                mu = {S: mono[:, S, g0:RG].bitcast(U32) for S in range(8)}
                sy = gyh[:, g0:RG].bitcast(U32)
                sx = gxh[:, g0:RG].bitcast(U32)
                sd = eh[:, g0:RG].bitcast(U32)
                mk = maskT[:, 0:1]
                stt = nc.vector.scalar_tensor_tensor
                stt(mu[2], sx, mk, mu[0], ALU.bitwise_and, ALU.bitwise_xor)
                stt(mu[4], sy, mk, mu[0], ALU.bitwise_and, ALU.bitwise_xor)
                stt(mu[6], sy, mk, mu[2], ALU.bitwise_and, ALU.bitwise_xor)
                stt(mu[1], sd, mk, mu[0], ALU.bitwise_and, ALU.bitwise_xor)
                stt(mu[3], sd, mk, mu[2], ALU.bitwise_and, ALU.bitwise_xor)
                stt(mu[5], sd, mk, mu[4], ALU.bitwise_and, ALU.bitwise_xor)
                stt(mu[7], sd, mk, mu[6], ALU.bitwise_and, ALU.bitwise_xor)


                # ---- conv matmuls: m-outer so each monomial gets two slots
                # of PE runway before the next one is needed; 4-way PE tile
                # packing per slot.  Monomials 5,7: taps 6-8 in fp16, taps 0-5
                # as fp8 DoubleRow pairs (2 taps' worth of MACs per cycle).
                first = True
                plan = [("f16", m, tap) for m in MORDER for tap in range(9)]
                for pi, (kind, m, tap) in enumerate(plan):
                    dy, dx = tap // 3, tap % 3
                    st = (pi == len(plan) - 1)
                    for sj in range(R // 4):
                        ps = pss[(2 * ci + sj) % 4]
                        ps_t = ps[:, 0:512]
                        ps_b = ps[:, 512:1024]
                        rA = 4 * sj + dy
                        rB = rA + 2
                        for (pr, psq, rr) in ((0, ps_t, rA), (64, ps_b, rA),
                                              (0, ps_t, rB), (64, ps_b, rB)):
                            pc = 0 if rr == rA else 64
                            nc.tensor.matmul(
                                psq[pc:pc + 64, :],
                                wt[pr:pr + 64, m, tap, :],
                                mono[pr:pr + 64, m, rr:rr + 2, dx:dx + W_],
                                start=first, stop=st,
                                skip_group_check=True,
                            )
                    first = False
                for sj in range(R // 4):
                    pending.append((pss[(2 * ci + sj) % 4], r0 + 4 * sj))
                if ci == NCHUNK - 1:
                    for _ in range(2):
                        ps, y0 = pending.pop(0)
                        evac(ps, y0)

            for ps, y0 in pending:
                evac(ps, y0)

    nc.compile()
    return nc


SCALE = 256.0


def _prep_host_inputs(Wfull: np.ndarray):
    """Monomial weights wt[128,8,9,O] fp16 (x SCALE; undone on host)."""
    sig = np.zeros((K, 3), np.float64)
    for k in range(K):
        a_, b_, c_ = (k >> 2) & 1, (k >> 1) & 1, k & 1
        Sy, Sx, D = a_, a_ ^ b_, b_ ^ c_
        sig[k] = [2 * Sy - 1, 2 * Sx - 1, 2 * D - 1]
    Wd = Wfull.astype(np.float64)  # (K, O, C, 3, 3)
    wt = np.zeros((64, 8, 9, O), np.float64)
    for S in range(8):
        coef = np.ones(K)
        if S & 4: coef = coef * sig[:, 0]
        if S & 2: coef = coef * sig[:, 1]
        if S & 1: coef = coef * sig[:, 2]
        Wp = np.einsum('k,kocyx->ocyx', coef, Wd) / 64.0  # (O, C, 3, 3)
        wt[:, S, :, :] = np.transpose(Wp.reshape(O, C, 9), (1, 2, 0))
    wt = wt * SCALE
    wt128 = np.concatenate([wt, wt], axis=0).astype(np.float16)
    return wt128


_NC_CACHE = None


def _get_nc():
    global _NC_CACHE
    if _NC_CACHE is None:
        _NC_CACHE = _build_nc()
    return _NC_CACHE


LAST_RESULT = None


def kernel(x: np.ndarray, W: np.ndarray, b: np.ndarray, **run_kwargs) -> np.ndarray:
    global LAST_RESULT
    assert x.shape == (B, C, H, W_) and W.shape == (K, O, C, 3, 3)
    nc = _get_nc()
    wt128 = _prep_host_inputs(np.asarray(W))
    xs = np.ascontiguousarray(np.asarray(x, dtype=np.float32))
    in_maps = [
        {"x": xs[i], "wt": wt128}
        for i in range(B)
    ]
    res = bass_utils.run_bass_kernel_spmd(nc, in_maps, core_ids=list(range(B)),
                                          **run_kwargs)
    LAST_RESULT = res
    out = np.stack([res.results[i]["out"] for i in range(B)], axis=0)
    out = out.astype(np.float32) / SCALE
    # bias (mean over k) is a per-channel constant: add on host
    bias = (np.asarray(b).astype(np.float64).sum(axis=0) / K).astype(np.float32)
    out = out + bias[None, :, None, None]
    return out.astype(np.float32)


if __name__ == "__main__":
    nc = _get_nc()
    print("built + compiled OK")


# revision 16
# speedup vs baseline: 1.1999x; 1.1999x over previous
"""Trainium2 Bass kernel for nn_DirectionalConv (moe_routing).

Math: out = (1/8) * sum_k conv3x3(x * [octant(sobel(x)) == k], W[k]) + mean_k b[k]

Implementation notes:
- Data-parallel over batch B=8 across 8 NeuronCores (one image per core).
- Octant selection is rewritten in a +-1 "monomial" basis over the three sign
  bits (sign(gy), sign(gx), sign(|gy|-|gx|)):
      sum_k conv(x*mask_k, W[k]) = sum_{S in 2^3} conv(x*chi_S, W'_S)
  where chi_S = product of the selected signs (computed with pure bitwise
  XOR of sign bits - exact) and W'_S = (1/64) sum_k chi_S(k) W[k] is
  precomputed on the host.  This gives 8 dense 3x3 convs, evaluated as
  9 shifted matmuls each, accumulating in PSUM.
- Per-core image (64,256,256) is split into top/bottom halves across the
  SBUF partition dimension: partition p = (half<<6)|channel.  Conv matmuls
  are K=64 and use 4-way PE tile packing (2 row-groups x 2 col-groups) to
  fill the 128x128 array.
- PSUM is DMAed straight to HBM (bias is added on the host), so neither ACT
  nor DVE ever waits on PE progress; out-DMAs are emitted one chunk late so
  no DMA descriptor ever parks in a queue blocking the x prefetch.
- Emission is software-pipelined: at/ut/mono0 of chunk i+1 are emitted in
  body i so the DVE FIFO has fill work while ACT produces the sign casts.
- Gradient tensors' border columns are never zeroed: only their sign bit is
  consumed, and it only flips the sign of an (exactly zero) padding value.
"""

import numpy as np

import concourse.bacc as bacc
import concourse.bass as bass
import concourse.mybir as mybir
from concourse import bass_utils
from concourse.tile import TileContext

F32 = mybir.dt.float32
F16 = mybir.dt.float16
U32 = mybir.dt.uint32
ALU = mybir.AluOpType
ACTF = mybir.ActivationFunctionType

B, C, H, W_, K, O = 8, 64, 256, 256, 8, 64
HH = H // 2          # rows per half
R = 8                # output rows per half per chunk
NCHUNK = HH // R     # 16
RG = R + 2           # gradient rows per chunk
WP = W_ + 2          # padded width 258
SIGN16 = 0x80008000  # sign bits of two packed fp16 lanes

MORDER = (0, 2, 4, 6, 1, 3, 5, 7)  # monomial readiness order


def _build_nc():
    nc = bacc.Bacc("TRN2", target_bir_lowering=False, debug=False)

    x_d = nc.dram_tensor("x", [C, H, W_], F32, kind="ExternalInput")
    wt_d = nc.dram_tensor("wt", [128, 8, 9, O], F16, kind="ExternalInput")
    out_d = nc.dram_tensor("out", [O, H, W_], F16, kind="ExternalOutput")

    with TileContext(nc) as tc:
        with (
            tc.tile_pool(name="sb", bufs=1) as sb,
            tc.tile_pool(name="pp", bufs=1, space="PSUM") as pp,
        ):
            # ---- static tiles
            wt = sb.tile([128, 8, 9, O], F16, tag="wt")
            maskT = sb.tile([128, 1], U32, tag="maskT")
            nc.gpsimd.memset(maskT[:], SIGN16)

            # ---- rotating buffers (explicit, so border memsets hoist)
            xts = [sb.tile([128, R + 4, WP], F32, tag=f"xt{i}", name=f"xt{i}")
                   for i in range(3)]
            for xt in xts:
                nc.gpsimd.memset(xt[:, :, 0:1], 0.0)
                nc.gpsimd.memset(xt[:, :, WP - 1:WP], 0.0)
            monos = [sb.tile([128, 8, RG, WP], F16, tag=f"mono{i}",
                             name=f"mono{i}") for i in range(2)]
            at_t = sb.tile([128, RG, WP], F32, tag="at")
            tt_t = sb.tile([128, RG, WP], F32, tag="tt")
            ut_t = sb.tile([128, RG, WP], F32, tag="ut")
            gx_t = sb.tile([128, RG, WP], F32, tag="gx")
            gy_t = sb.tile([128, RG, WP], F32, tag="gy")
            ax_t = sb.tile([128, RG, W_], F32, tag="ax")
            gxh = sb.tile([128, RG, WP], F16, tag="gxh")
            gyh = sb.tile([128, RG, WP], F16, tag="gyh")
            eh = sb.tile([128, RG, WP], F16, tag="eh")
            stgs = [sb.tile([128, 1024], F16, tag=f"stg{i}", name=f"stg{i}")
                    for i in range(2)]
            pss = [pp.tile([128, 1024], F32, tag=f"ps{i}", name=f"ps{i}")
                   for i in range(4)]

            def dma_x(ci):
                xt = xts[ci % 3]
                r0 = ci * R
                tlo, thi = r0 - 2, r0 + R + 2
                if tlo < 0:
                    nc.gpsimd.memset(xt[0:64, 0:-tlo, 1:WP - 1], 0.0)
                    nc.sync.dma_start(xt[0:64, -tlo:R + 4, 1:WP - 1],
                                      x_d[:, 0:thi, :])
                else:
                    nc.sync.dma_start(xt[0:64, :, 1:WP - 1], x_d[:, tlo:thi, :])
                blo, bhi = HH + r0 - 2, HH + r0 + R + 2
                if bhi > H:
                    nval = H - blo
                    nc.gpsimd.memset(xt[64:128, nval:R + 4, 1:WP - 1], 0.0)
                    nc.sync.dma_start(xt[64:128, 0:nval, 1:WP - 1],
                                      x_d[:, blo:H, :])
                else:
                    nc.sync.dma_start(xt[64:128, :, 1:WP - 1], x_d[:, blo:bhi, :])

            def lead_ops(ci):
                """chunk ci's chain head: emitted one body early (sw pipeline).
                All xt readers live here so the next x prefetch DMA's WAR
                releases a full chunk early.  For ci>0, gradient/monomial rows
                0..1 equal the previous chunk's rows 8..9 (same image rows):
                they are carried over by one ACT copy and only rows 2.. are
                computed."""
                xt = xts[ci % 3]
                g0 = 0 if ci == 0 else 2
                nc.vector.tensor_add(at_t[:, g0:RG], xt[:, g0:RG, :],
                                     xt[:, g0 + 2:RG + 2, :])
                nc.vector.tensor_sub(ut_t[:, g0:RG], xt[:, g0:RG, :],
                                     xt[:, g0 + 2:RG + 2, :])
                nc.vector.scalar_tensor_tensor(tt_t[:, g0:RG],
                                               xt[:, g0 + 1:RG + 1, :], 2.0,
                                               at_t[:, g0:RG], ALU.mult, ALU.add)
                nc.scalar.activation(monos[ci % 2][:, 0, g0:RG],
                                     xt[:, g0 + 1:RG + 1, :], ACTF.Copy)

            def evac(ps, y0):
                # ACT-issued DMAs (the second hardware DGE queue): the x
                # prefetch queue (SP) must never park behind PE-gated
                # descriptors, or the whole pipeline latency-locks.
                stg = stgs[(y0 // 4) % 2]
                nc.scalar.activation(stg[:], ps[:], ACTF.Copy)
                yb = HH + y0
                nc.gpsimd.dma_start(out_d[:, y0:y0 + 2, :], stg[0:64, 0:512])
                nc.gpsimd.dma_start(out_d[:, y0 + 2:y0 + 4, :], stg[64:128, 0:512])
                nc.gpsimd.dma_start(out_d[:, yb:yb + 2, :], stg[0:64, 512:1024])
                nc.gpsimd.dma_start(out_d[:, yb + 2:yb + 4, :], stg[64:128, 512:1024])

            dma_x(0)
            nc.sync.dma_start(wt[:], wt_d[:])
            dma_x(1)

            # ---- PE warm-up: dummy matmuls on garbage (stg) data into the
            # first PSUM tile keep the PE busy through the HAM activity
            # window while the first x DMA lands, so real work starts at
            # 2.4 GHz.  Chunk 0's start=True matmuls overwrite the bank.
            for wi in range(28):
                nc.tensor.matmul(
                    pss[3][0:64, 0:512],
                    stgs[0][0:64, 0:64],
                    stgs[1][0:64, 0:512],
                    start=(wi == 0), stop=(wi == 27),
                    skip_group_check=True,
                )

            lead_ops(0)
            pending = []  # 2-chunk-delayed PSUM evacuations: (ps, y0)

            for ci in range(NCHUNK):
                if ci + 2 < NCHUNK:
                    dma_x(ci + 2)
                xt = xts[ci % 3]
                mono = monos[ci % 2]
                r0 = ci * R

                # ---- evacuate the 2-chunks-ago PSUM banks at the body top:
                # by the time ACT reaches these in FIFO order their PE deps
                # are long satisfied, and PE's bank reuse (WAR) is met a full
                # chunk before it matters.
                if ci >= 2:
                    for _ in range(2):
                        ps, y0 = pending.pop(0)
                        evac(ps, y0)

                g0 = 0 if ci == 0 else 2
                # ---- carry monomial rows 0..1 from the previous chunk's
                # buffer (same image rows as its rows 8..9): one ACT copy for
                # all 8 monomials.
                if ci > 0:
                    nc.scalar.activation(mono[:, :, 0:2, :],
                                         monos[(ci - 1) % 2][:, :, R:RG, :],
                                         ACTF.Copy)

                # ---- gx chain (DVE + ACT cast)
                nc.vector.tensor_tensor(gx_t[:, g0:RG, 1:WP - 1],
                                        tt_t[:, g0:RG, 0:WP - 2],
                                        tt_t[:, g0:RG, 2:WP], ALU.subtract)
                nc.scalar.activation(gxh[:, g0:RG], gx_t[:, g0:RG], ACTF.Copy)

                # ---- gy chain; b2 reuses at's memory (at consumed by tt)
                b2 = at_t[:, g0:RG, 0:W_]
                nc.vector.tensor_add(b2, ut_t[:, g0:RG, 0:WP - 2],
                                     ut_t[:, g0:RG, 2:WP])
                nc.vector.scalar_tensor_tensor(gy_t[:, g0:RG, 1:WP - 1],
                                               ut_t[:, g0:RG, 1:WP - 1], 2.0, b2,
                                               ALU.mult, ALU.add)
                nc.scalar.activation(gyh[:, g0:RG], gy_t[:, g0:RG], ACTF.Copy)

                # ---- |gy|, |gx| on ACT; ay reuses tt's memory
                ay = tt_t[:, g0:RG, 0:W_]
                nc.scalar.activation(ay, gy_t[:, g0:RG, 1:WP - 1], ACTF.Abs)
                nc.scalar.activation(ax_t[:, g0:RG, :], gx_t[:, g0:RG, 1:WP - 1],
                                     ACTF.Abs)

                # ---- e = |gy|-|gx| into gx's memory (only sign bits survive)
                nc.vector.tensor_tensor(gx_t[:, g0:RG, 1:WP - 1], ay,
                                        ax_t[:, g0:RG], ALU.subtract)
                nc.scalar.activation(eh[:, g0:RG], gx_t[:, g0:RG], ACTF.Copy)

                # ---- next chunk's chain head: emitted after e32 so the WAR on
                # ay (living in tt's tile) is tracked; the engine wait-queue
                # lets these bypass a parked e32 and fill the DVE bubble.
                if ci + 1 < NCHUNK:
                    lead_ops(ci + 1)

                # ---- monomials y_S = x * chi_S, S = (sy<<2)|(sx<<1)|sd
                mu = {S: mono[:, S, g0:RG].bitcast(U32) for S in range(8)}
                sy = gyh[:, g0:RG].bitcast(U32)
                sx = gxh[:, g0:RG].bitcast(U32)
                sd = eh[:, g0:RG].bitcast(U32)
                mk = maskT[:, 0:1]
                stt = nc.vector.scalar_tensor_tensor
                stt(mu[2], sx, mk, mu[0], ALU.bitwise_and, ALU.bitwise_xor)
                stt(mu[4], sy, mk, mu[0], ALU.bitwise_and, ALU.bitwise_xor)
                stt(mu[6], sy, mk, mu[2], ALU.bitwise_and, ALU.bitwise_xor)
                stt(mu[1], sd, mk, mu[0], ALU.bitwise_and, ALU.bitwise_xor)
                stt(mu[3], sd, mk, mu[2], ALU.bitwise_and, ALU.bitwise_xor)
                stt(mu[5], sd, mk, mu[4], ALU.bitwise_and, ALU.bitwise_xor)
                stt(mu[7], sd, mk, mu[6], ALU.bitwise_and, ALU.bitwise_xor)

                # ---- conv matmuls: m-outer so each monomial gets two slots
                # of PE runway before the next one is needed; 4-way PE tile
                # packing per slot.
                first = True
                for m in MORDER:
                    for tap in range(9):
                        dy, dx = tap // 3, tap % 3
                        st = (m == MORDER[-1] and tap == 8)
                        for sj in range(R // 4):
                            ps = pss[(2 * ci + sj) % 4]
                            ps_t = ps[:, 0:512]
                            ps_b = ps[:, 512:1024]
                            rA = 4 * sj + dy
                            rB = rA + 2
                            for (pr, psq, rr) in ((0, ps_t, rA), (64, ps_b, rA),
                                                  (0, ps_t, rB), (64, ps_b, rB)):
                                pc = 0 if rr == rA else 64
                                nc.tensor.matmul(
                                    psq[pc:pc + 64, :],
                                    wt[pr:pr + 64, m, tap, :],
                                    mono[pr:pr + 64, m, rr:rr + 2, dx:dx + W_],
                                    start=first, stop=st,
                                    skip_group_check=True,
                                )
                        first = False
                for sj in range(R // 4):
                    pending.append((pss[(2 * ci + sj) % 4], r0 + 4 * sj))
                if ci == NCHUNK - 1:
                    for _ in range(2):
                        ps, y0 = pending.pop(0)
                        evac(ps, y0)

            for ps, y0 in pending:
                evac(ps, y0)

    nc.compile()
    return nc


def _prep_host_inputs(Wfull: np.ndarray):
    """Monomial weights wt[128, 8, 9, O] fp16."""
    sig = np.zeros((K, 3), np.float64)
    for k in range(K):
        a_, b_, c_ = (k >> 2) & 1, (k >> 1) & 1, k & 1
        Sy, Sx, D = a_, a_ ^ b_, b_ ^ c_
        sig[k] = [2 * Sy - 1, 2 * Sx - 1, 2 * D - 1]
    Wd = Wfull.astype(np.float64)  # (K, O, C, 3, 3)
    wt = np.zeros((64, 8, 9, O), np.float64)
    for S in range(8):
        coef = np.ones(K)
        if S & 4: coef = coef * sig[:, 0]
        if S & 2: coef = coef * sig[:, 1]
        if S & 1: coef = coef * sig[:, 2]
        Wp = np.einsum('k,kocyx->ocyx', coef, Wd) / 64.0  # (O, C, 3, 3)
        wt[:, S, :, :] = np.transpose(Wp.reshape(O, C, 9), (1, 2, 0))
    wt128 = np.concatenate([wt, wt], axis=0).astype(np.float16)
    return wt128


_NC_CACHE = None


def _get_nc():
    global _NC_CACHE
    if _NC_CACHE is None:
        _NC_CACHE = _build_nc()
    return _NC_CACHE


LAST_RESULT = None


def kernel(x: np.ndarray, W: np.ndarray, b: np.ndarray, **run_kwargs) -> np.ndarray:
    global LAST_RESULT
    assert x.shape == (B, C, H, W_) and W.shape == (K, O, C, 3, 3)
    nc = _get_nc()
    wt128 = _prep_host_inputs(np.asarray(W))
    xs = np.ascontiguousarray(np.asarray(x, dtype=np.float32))
    in_maps = [
        {"x": xs[i], "wt": wt128}
        for i in range(B)
    ]
    res = bass_utils.run_bass_kernel_spmd(nc, in_maps, core_ids=list(range(B)),
                                          **run_kwargs)
    LAST_RESULT = res
    out = np.stack([res.results[i]["out"] for i in range(B)], axis=0)
    out = out.astype(np.float32)
    # bias (mean over k) is a per-channel constant: add on host
    bias = (np.asarray(b).astype(np.float64).sum(axis=0) / K).astype(np.float32)
    out = out + bias[None, :, None, None]
    return out.astype(np.float32)


if __name__ == "__main__":
    nc = _get_nc()
    print("built + compiled OK")
